# revision 1
# baseline (speedup 1.0000x reference)
"""MultiHeadCrossAttention Trainium2 kernel (8 NeuronCores, SPMD).

Sharding: core c -> (n = c // 2, g = c % 2). Each core handles one query
batch n and half the heads (8 of 16, embed slice g*512:(g+1)*512).

Host side: transpose queries/keys/values into [dim, tokens] layout (the
TensorEngine contracts along the partition dim, so both matmul operands
need the contraction dim on partitions), compact keys/values along KLEN
by the per-n mask (~50% survive), pad to KC = 128*T, cast to bf16.
The unnormalized AV outputs and softmax denominators come back per core;
the host divides while assembling/transposing the full output.

Device side per core (all matmuls bf16, fp32 PSUM accumulation):
  - qT/kT projections in transposed layout (lhsT = W chunk, rhs = xT);
    kT lands in per-head zero-padded slots (kTz) so the energy matmuls
    run with K=128 - full PE-array activity keeps the HAM clock at
    2.4 GHz (K=64 matmuls measurably re-throttle the PE to 1.2 GHz).
  - v projection in [k, emb] layout (lhsT = vT k-tile, rhs = W chunk).
  - energyT[k, q] = kTz.T @ qT per head, one PSUM bank per (head, k-tile).
  - exp on ScalarE (scale=1/8) PSUM->SBUF bf16, batched 2 k-tiles per
    ACTIVATE. This is the kernel bottleneck: softmax exp runs at
    1 elem/cycle/lane at 1.2 GHz and only ScalarE can do it.
  - AV with lhsT = [v_h | valid-indicator | filler] (M=128) accumulated
    over k-tiles into one PSUM bank per head: row 64 of the accumulator
    is the softmax denominator for free. Padded k rows have v=0 and
    indicator=0 so they contribute nothing anywhere.
  - software pipeline: energy of item i+1 runs on the PE while exp of
    item i streams on ScalarE (across q-chunk boundaries too);
    projections of head-pair c+1 overlap the exp-bound attention of
    pair c; junk matmuls during the input-DMA window pre-warm the PE
    clock gate.
"""

import math
import sys
from contextlib import ExitStack

import numpy as np

for _p in ("/opt/trn_rl_repo",):
    if _p not in sys.path:
        sys.path.insert(0, _p)

import ml_dtypes

import concourse.bass as bass  # noqa: F401  (import registers lowering deps)
import concourse.tile as tile
from concourse import bacc, mybir
from concourse.bass_utils import run_bass_kernel_spmd

BF16 = ml_dtypes.bfloat16

N, QLEN, KLEN = 4, 2048, 2048
QDIM = KVDIM = 512
EMBED, HEADS = 1024, 16
HEAD_DIM = 64
N_CORES = 8
QCH = 512  # q-chunk width (one PSUM bank of fp32)
SCALE = 1.0 / math.sqrt(HEAD_DIM)

_cache: dict = {}
last_exec_time_ns = None
last_results = None


def _build(T: int, ql: int = QLEN):
    """Build the per-core Bass program for KC = 128*T compacted kv tokens."""
    KC = 128 * T
    dt = mybir.dt
    nc = bacc.Bacc("TRN2", target_bir_lowering=False, debug=False)

    qT_d = nc.dram_tensor("qt", [QDIM, ql], dt.bfloat16, kind="ExternalInput").ap()
    kT_d = nc.dram_tensor("kt", [KVDIM, KC], dt.bfloat16, kind="ExternalInput").ap()
    vT_d = nc.dram_tensor("vt", [KVDIM, KC], dt.bfloat16, kind="ExternalInput").ap()
    wq_d = nc.dram_tensor("wq", [QDIM, 512], dt.bfloat16, kind="ExternalInput").ap()
    wk_d = nc.dram_tensor("wk", [KVDIM, 512], dt.bfloat16, kind="ExternalInput").ap()
    wv_d = nc.dram_tensor("wv", [KVDIM, 512], dt.bfloat16, kind="ExternalInput").ap()
    # per-row validity indicator (1.0 real kv token, 0.0 pad), [128, T]
    vind_d = nc.dram_tensor("vind", [128, T], dt.float32, kind="ExternalInput").ap()
    # rows (c*2+h)*65 .. +64: unnormalized AV.T ; row +64: denominator
    out_d = nc.dram_tensor("out", [520, ql], dt.float32, kind="ExternalOutput").ap()

    NQ = ql // QCH
    kcols = [(s, min(512, KC - s)) for s in range(0, KC, 512)]
    w_dram = {"wq": wq_d, "wk": wk_d, "wv": wv_d}
    # k-tile groups of 2 for batched exp (PSUM tile = 2 banks)
    groups = [tuple(range(t, min(t + 2, T))) for t in range(0, T, 2)]

    with tile.TileContext(nc) as tc:
        with ExitStack() as ctx:
            persist = ctx.enter_context(tc.tile_pool(name="persist", bufs=1))

            qTin = [persist.tile([128, ql], dt.bfloat16, tag=f"qTin{j}", name=f"qTin{j}") for j in range(4)]
            kTin = [persist.tile([128, KC], dt.bfloat16, tag=f"kTin{j}", name=f"kTin{j}") for j in range(4)]
            vTin = [persist.tile([128, KC], dt.bfloat16, tag=f"vTin{j}", name=f"vTin{j}") for j in range(4)]
            wsb = {
                nm: [persist.tile([128, 512], dt.bfloat16, tag=f"{nm}{j}", name=f"{nm}{j}") for j in range(4)]
                for nm in ("wq", "wk", "wv")
            }
            qT = [persist.tile([128, ql], dt.bfloat16, tag=f"qT{c}", name=f"qT{c}") for c in range(4)]
            # kTz[c][:, h, :]: rows h*64..h*64+63 hold head h's kT rows, the
            # other 64 rows stay zero -> energy matmuls run with K=128 (full
            # PE array activity) at the same stream cost.
            kTz = [persist.tile([128, 2, KC], dt.bfloat16, tag=f"kTz{c}", name=f"kTz{c}") for c in range(4)]
            # [v_h (64) | indicator (1) | filler (63)]: M=128 keeps the full
            # array busy; output rows 65-127 are ignored.
            vsb = persist.tile([128, T, 4, 2, 128], dt.bfloat16, tag="v", name="v")
            vind = persist.tile([128, T], dt.float32, tag="vind", name="vind")

            nc.gpsimd.memset(vsb, 1.0)
            for c in range(4):
                nc.gpsimd.memset(kTz[c], 0.0)
            for nm in ("wq", "wk", "wv"):
                for j in range(4):
                    nc.sync.dma_start(wsb[nm][j], w_dram[nm][j * 128:(j + 1) * 128, :])
            for j in range(4):
                nc.sync.dma_start(qTin[j], qT_d[j * 128:(j + 1) * 128, :])
            for j in range(4):
                nc.sync.dma_start(kTin[j], kT_d[j * 128:(j + 1) * 128, :])
            for j in range(4):
                nc.sync.dma_start(vTin[j], vT_d[j * 128:(j + 1) * 128, :])
            nc.sync.dma_start(vind, vind_d)
            # validity indicator column for every k-tile (pads can span more
            # than one trailing tile when counts differ across cores)
            for t in range(T):
                for c in range(4):
                    for h in range(2):
                        nc.vector.tensor_copy(vsb[:, t, c, h, 64:65], vind[:, t:t + 1])

            # ---- interleaved: per head-pair c, project then attend ----
            # PSUM budget: psA 2 banks (bufs=1) + e 4 (bufs=2 x 2 banks) +
            # av 2 (bufs=1 x 2 tags) = 8. Projections for pair c+1 overlap
            # the exp-bound attention of pair c on the TensorEngine.
            with tc.tile_pool(name="psA", bufs=1, space="PSUM") as psA, \
                 tc.tile_pool(name="psE", bufs=2, space="PSUM") as psE, \
                 tc.tile_pool(name="psO", bufs=1, space="PSUM") as psO, \
                 tc.tile_pool(name="sbx", bufs=4) as sbx, \
                 tc.tile_pool(name="sbo", bufs=3) as sbo:
                junk = persist.tile([128, 512], dt.bfloat16, tag="junk", name="junk")
                nc.vector.memset(junk, 1.0)
                for _ in range(4):
                    ps = psA.tile([128, QCH], dt.float32, tag="pA", name="pA")
                    for r in range(10):
                        nc.tensor.matmul(ps, lhsT=junk[:, :128], rhs=junk,
                                         start=(r == 0), stop=(r == 9))

                for c in range(4):
                    for q0 in range(NQ):
                        ps = psA.tile([128, QCH], dt.float32, tag="pA", name="pA")
                        for j in range(4):
                            nc.tensor.matmul(
                                ps,
                                lhsT=wsb["wq"][j][:, c * 128:(c + 1) * 128],
                                rhs=qTin[j][:, q0 * QCH:(q0 + 1) * QCH],
                                start=(j == 0), stop=(j == 3),
                            )
                        nc.vector.tensor_copy(qT[c][:, q0 * QCH:(q0 + 1) * QCH], ps)
                    for (s, w) in kcols:
                        ps = psA.tile([128, QCH], dt.float32, tag="pA", name="pA")
                        for j in range(4):
                            nc.tensor.matmul(
                                ps[:, :w],
                                lhsT=wsb["wk"][j][:, c * 128:(c + 1) * 128],
                                rhs=kTin[j][:, s:s + w],
                                start=(j == 0), stop=(j == 3),
                            )
                        nc.vector.tensor_copy(kTz[c][0:64, 0, s:s + w], ps[0:64, :w])
                        nc.vector.tensor_copy(kTz[c][64:128, 1, s:s + w], ps[64:128, :w])
                    for t in range(T):
                        ps = psA.tile([128, 128], dt.float32, tag="pAv", name="pAv")
                        for j in range(4):
                            nc.tensor.matmul(
                                ps,
                                lhsT=vTin[j][:, t * 128:(t + 1) * 128],
                                rhs=wsb["wv"][j][:, c * 128:(c + 1) * 128],
                                start=(j == 0), stop=(j == 3),
                            )
                        nc.vector.tensor_copy(vsb[:, t, c, 0, 0:64], ps[:, 0:64])
                        nc.vector.tensor_copy(vsb[:, t, c, 1, 0:64], ps[:, 64:128])

                    def emit_energy(q0, grp):
                        eh = [psE.tile([128, 2 * QCH], dt.float32, tag="e", name="e")
                              for _ in range(2)]
                        for h in range(2):
                            for b, t in enumerate(grp):
                                nc.tensor.matmul(
                                    eh[h][:, b * QCH:(b + 1) * QCH],
                                    lhsT=kTz[c][:, h, t * 128:(t + 1) * 128],
                                    rhs=qT[c][:, q0 * QCH:(q0 + 1) * QCH],
                                    start=True, stop=True,
                                )
                        return eh

                    def emit_exp_av(q0, grp, eh, av):
                        gw = len(grp) * QCH
                        ex = [sbx.tile([128, 2 * QCH], dt.bfloat16, tag="x", name="x")
                              for _ in range(2)]
                        for h in range(2):
                            nc.scalar.activation(
                                ex[h][:, :gw], eh[h][:, :gw],
                                mybir.ActivationFunctionType.Exp,
                                scale=SCALE,
                            )
                        for h in range(2):
                            for b, t in enumerate(grp):
                                nc.tensor.matmul(
                                    av[h],
                                    lhsT=vsb[:, t, c, h, :],
                                    rhs=ex[h][:, b * QCH:(b + 1) * QCH],
                                    start=(t == 0), stop=(t == T - 1),
                                )

                    def emit_out(q0, av):
                        for h in range(2):
                            ot = sbo.tile([65, QCH], dt.float32, tag="ot", name="ot")
                            nc.vector.tensor_copy(ot, av[h][0:65, :])
                            nc.sync.dma_start(
                                out_d[(c * 2 + h) * 65:(c * 2 + h) * 65 + 65,
                                      q0 * QCH:(q0 + 1) * QCH], ot)

                    # software pipeline over the flattened (q0, group) stream:
                    # energy of item i+1 runs on the PE while exp of item i
                    # streams on the ScalarE, including across q0 boundaries.
                    prev = None
                    for q0 in range(NQ):
                        av = [psO.tile([128, QCH], dt.float32, tag=f"av{h}",
                                       name=f"av{h}") for h in range(2)]
                        for grp in groups:
                            eh = emit_energy(q0, grp)
                            if prev is not None:
                                emit_exp_av(*prev)
                                if prev[1] is groups[-1]:
                                    emit_out(prev[0], prev[3])
                            prev = (q0, grp, eh, av)
                    emit_exp_av(*prev)
                    emit_out(prev[0], prev[3])

    nc.compile()
    return nc


def _prepare(queries, keys, values, mask):
    """Host-side sharding: transpose, compact kv by mask, validity tiles."""
    m = np.asarray(mask).reshape(N, KLEN) != 0
    idx = [np.nonzero(m[n])[0] for n in range(N)]
    cnts = [len(i) for i in idx]
    T = max(1, (max(cnts) + 127) // 128)
    KC = 128 * T

    kT_full = np.ascontiguousarray(np.asarray(keys, np.float32)[0].T)
    vT_full = np.ascontiguousarray(np.asarray(values, np.float32)[0].T)
    q32 = np.asarray(queries, np.float32)

    qT_n, kT_n, vT_n, vind_n = [], [], [], []
    for n in range(N):
        kt = np.zeros((KVDIM, KC), np.float32)
        vt = np.zeros((KVDIM, KC), np.float32)
        kt[:, :cnts[n]] = kT_full[:, idx[n]]
        vt[:, :cnts[n]] = vT_full[:, idx[n]]
        ind = (np.arange(KC) < cnts[n]).astype(np.float32)
        vind_n.append(np.ascontiguousarray(ind.reshape(T, 128).T))
        kT_n.append(kt.astype(BF16))
        vT_n.append(vt.astype(BF16))
        qT_n.append(np.ascontiguousarray(q32[n].T).astype(BF16))
    return T, qT_n, kT_n, vT_n, vind_n


def kernel(queries, keys, values, mask, Wq, Wk, Wv, _trace=False):
    global last_exec_time_ns, last_results
    T, qT_n, kT_n, vT_n, vind_n = _prepare(queries, keys, values, mask)

    w_g = {}
    for nm, W in (("wq", Wq), ("wk", Wk), ("wv", Wv)):
        W = np.asarray(W, np.float32)
        w_g[nm] = [np.ascontiguousarray(W[:, g * 512:(g + 1) * 512]).astype(BF16)
                   for g in range(2)]

    nc = _cache.get(T)
    if nc is None:
        nc = _cache.setdefault(T, _build(T))

    in_maps = []
    for core in range(N_CORES):
        n, g = core // 2, core % 2
        in_maps.append({
            "qt": qT_n[n], "kt": kT_n[n], "vt": vT_n[n],
            "wq": w_g["wq"][g], "wk": w_g["wk"][g], "wv": w_g["wv"][g],
            "vind": vind_n[n],
        })

    res = run_bass_kernel_spmd(nc, in_maps, core_ids=list(range(N_CORES)),
                               trace=bool(_trace))
    last_exec_time_ns = res.exec_time_ns
    last_results = res

    full = np.empty((N, QLEN, EMBED), np.float32)
    for core in range(N_CORES):
        n, g = core // 2, core % 2
        o = res.results[core]["out"].reshape(8, 65, QLEN)
        vals = o[:, :64, :] / o[:, 64:65, :]          # [8, 64, QLEN]
        full[n, :, g * 512:(g + 1) * 512] = (
            vals.transpose(2, 0, 1).reshape(QLEN, 512)
        )
    return full

